# revision 1
# baseline (speedup 1.0000x reference)
"""3-layer GraphSAGE (mean agg) on 8 trn2 NeuronCores.

Sharding: nodes data-parallel (6250/core, by dst owner), weights replicated.
Per core: edges with dst in its node range, sorted by (src-half, dst-block),
padded to core-invariant per-(block,run) window counts so one SPMD program
works for all cores. Neighbor features fetched by dma_gather (bf16 rows),
aggregated per 128-edge window by one-hot matmuls accumulating in PSUM,
then fused dense layer (W_self, W_neigh, bias via K=1 matmul) in fp32.
AllGather of bf16 activations between layers.
"""

import os
import sys

sys.path.insert(0, "/opt/trn_rl_repo")

import numpy as np
import ml_dtypes

N_NODES = 50000
N_EDGES = 800000
DIM = 128
N_LAYERS = 3
CORES = 8
NPC = N_NODES // CORES          # 6250 nodes per core
BLK = 128
NBLK = (NPC + BLK - 1) // BLK   # 49 blocks (last has 106 valid rows)
NPC_PAD = NBLK * BLK            # 6272
HALF = N_NODES // 2             # 25000 (int16 gather index limit is 32767)
GB = 5                          # blocks per gather group

LAST_EXEC_NS = [None]
LAST_PROFILE = [None]


def _preprocess(src, dst):
    """Host-side graph preprocessing. Returns per-core index/dslot arrays plus
    the shared (core-invariant) window schedule."""
    src = np.asarray(src).astype(np.int64)
    dst = np.asarray(dst).astype(np.int64)

    owner = dst // NPC
    run = (src >= HALF).astype(np.int64)
    dloc = dst - owner * NPC
    blk = dloc // BLK

    # counts[c, b, r]
    counts = np.zeros((CORES, NBLK, 2), np.int64)
    np.add.at(counts, (owner, blk, run), 1)
    # core-invariant window counts per (block, run)
    W = np.maximum(1, -(-counts.max(axis=0) // BLK))  # [NBLK, 2] ceil-div
    nw_run = W.sum(axis=0)                            # windows per run
    nw_total = int(nw_run.sum())

    per_core = []
    for c in range(CORES):
        sel = owner == c
        es, eb, er, ed = src[sel], blk[sel], run[sel], dloc[sel]
        order = np.lexsort((eb, er))
        es, eb, er, ed = es[order], eb[order], er[order], ed[order]

        idx_out = np.zeros(nw_total * BLK, np.int16)
        dslot_out = np.full(nw_total * BLK, 255.0, np.float32)
        wpos = 0
        epos = 0
        for r in range(2):
            for b in range(NBLK):
                cnt = int(counts[c, b, r])
                cap = int(W[b, r]) * BLK
                e0, e1 = epos, epos + cnt
                o0 = wpos * BLK
                idx_out[o0:o0 + cnt] = (es[e0:e1] - r * HALF).astype(np.int16)
                dslot_out[o0:o0 + cnt] = (ed[e0:e1] - b * BLK).astype(np.float32)
                # pads: idx 0 (valid row, gathered but zeroed by P)
                epos = e1
                wpos += int(W[b, r])
                assert cnt <= cap
        assert epos == len(es)

        # wrap indices in 16 partitions, replicate to 128 (one copy / Q7 core)
        wrapped = idx_out.reshape(-1, 16).T.copy()        # [16, nw_total*8]
        idx128 = np.tile(wrapped, (8, 1))                 # [128, nw_total*8]
        # dslotT: [128, nw_total]; column w = dslots of window w's 128 edges
        dslotT = dslot_out.reshape(nw_total, BLK).T.copy()
        per_core.append((idx128, dslotT))

    return W, per_core


def _build_program(W, nw_run):
    import concourse.bass as bass
    import concourse.mybir as mybir
    import concourse.tile as tile
    from concourse import bacc

    f32 = mybir.dt.float32
    bf16 = mybir.dt.bfloat16
    i16 = mybir.dt.int16

    nw_total = int(W.sum())

    nc = bacc.Bacc("TRN2", target_bir_lowering=False, num_devices=CORES)

    # I/O
    xbf = nc.declare_dram_parameter("xbf", [N_NODES, DIM], bf16, isOutput=False)
    xT_in = nc.declare_dram_parameter("xT", [DIM, NPC_PAD], f32, isOutput=False)
    idx_in = nc.declare_dram_parameter("idx", [128, nw_total * 8], i16, isOutput=False)
    dslot_in = nc.declare_dram_parameter("dslot", [128, nw_total], f32, isOutput=False)
    invdeg_in = nc.declare_dram_parameter("invdeg", [128, NPC_PAD], f32, isOutput=False)
    ws_in = nc.declare_dram_parameter("Wself", [N_LAYERS * DIM, DIM], f32, isOutput=False)
    wn_in = nc.declare_dram_parameter("Wneigh", [N_LAYERS * DIM, DIM], f32, isOutput=False)
    b_in = nc.declare_dram_parameter("bias", [N_LAYERS, DIM], f32, isOutput=False)
    iota_in = nc.declare_dram_parameter("iota", [128, 128], f32, isOutput=False)
    ident_in = nc.declare_dram_parameter("ident", [128, 128], f32, isOutput=False)
    out_ext = nc.declare_dram_parameter("out", [NPC, DIM], f32, isOutput=True)

    # internal DRAM for collectives
    hown = [nc.dram_tensor(f"hown{l}", [NPC, DIM], bf16) for l in range(2)]
    hg = [
        nc.dram_tensor(f"hg{l}", [N_NODES, DIM], bf16, addr_space="Shared")
        for l in range(2)
    ]
    rg = [list(range(CORES))]

    # group structure: groups of GB blocks
    groups = [list(range(s, min(s + GB, NBLK))) for s in range(0, NBLK, GB)]
    # per-run window offset of each (b, r) in the stream (run-major, block order)
    woff = np.zeros((NBLK, 2), np.int64)
    w = 0
    for r in range(2):
        for b in range(NBLK):
            woff[b, r] = w
            w += int(W[b, r])
    assert w == nw_total

    with tile.TileContext(nc) as tc:
        with (
            tc.tile_pool(name="persist", bufs=1) as pp,
            tc.tile_pool(name="msg", bufs=3) as msgp,
            tc.tile_pool(name="pwin", bufs=16) as pwp,
            tc.tile_pool(name="work", bufs=4) as wkp,
            tc.tile_pool(name="psA", bufs=2, space="PSUM") as psA,
            tc.tile_pool(name="psB", bufs=2, space="PSUM") as psB,
            tc.tile_pool(name="psT", bufs=2, space="PSUM") as psT,
        ):
            # --- persistent SBUF loads ---
            def load(shape, dt, src_ap, tag):
                t = pp.tile(shape, dt, tag=tag, name=tag)
                nc.sync.dma_start(out=t[:], in_=src_ap)
                return t

            idx_t = load([128, nw_total * 8], i16, idx_in[:, :], "idx")
            dslot_t = load([128, nw_total], f32, dslot_in[:, :], "dslot")
            invdeg_t = load([128, NPC_PAD], f32, invdeg_in[:, :], "invdeg")
            iota_t = load([128, 128], f32, iota_in[:, :], "iota")
            ident_t = load([128, 128], f32, ident_in[:, :], "ident")
            ws_t = [
                load([128, DIM], f32, ws_in[l * DIM : (l + 1) * DIM, :], f"ws{l}")
                for l in range(N_LAYERS)
            ]
            wn_t = [
                load([128, DIM], f32, wn_in[l * DIM : (l + 1) * DIM, :], f"wn{l}")
                for l in range(N_LAYERS)
            ]
            bias_t = [
                load([1, DIM], f32, b_in[l : l + 1, :], f"bias{l}")
                for l in range(N_LAYERS)
            ]
            ones_t = pp.tile([1, 128], f32, tag="ones", name="ones")
            nc.vector.memset(ones_t[:], 1.0)

            # h transposed (fp32) for the self path; ping-pong buffers
            hT = [
                load([DIM, NPC_PAD], f32, xT_in[:, :], "hT0"),
                pp.tile([DIM, NPC_PAD], f32, tag="hT1", name="hT1"),
            ]

            for l in range(N_LAYERS):
                src_dram = xbf if l == 0 else hg[l - 1]
                hT_cur = hT[l % 2]
                hT_next = hT[(l + 1) % 2]
                for g, blocks in enumerate(groups):
                    # two gathers (one per src half) covering this group's windows
                    msg_t = []
                    for r in range(2):
                        w0 = int(woff[blocks[0], r])
                        nw = int(sum(W[b, r] for b in blocks))
                        nidx = nw * BLK
                        mt = msgp.tile([128, nw * DIM], bf16, tag=f"msg{r}", name=f"msg{r}")
                        CH = 1024  # per-inst idx cap (SWDGE m2s/s2m rings = 1024 descs each)
                        for s0 in range(0, nidx, CH):
                            n = min(CH, nidx - s0)
                            nc.gpsimd.dma_gather(
                                out_ap=mt[:, s0 : s0 + n].rearrange(
                                    "p (w e) -> p w e", e=DIM
                                ),
                                in_ap=src_dram[r * HALF : (r + 1) * HALF, :],
                                idxs_ap=idx_t[
                                    :, w0 * 8 + s0 // 16 : w0 * 8 + (s0 + n) // 16
                                ],
                                num_idxs=n,
                                num_idxs_reg=n,
                                elem_size=DIM,
                                elem_step=DIM,
                            )
                        msg_t.append((mt, w0))

                    for b in blocks:
                        pa = psA.tile([128, 128], f32, tag="agg", name="agg")
                        nwin_b = int(W[b, 0] + W[b, 1])
                        wi = 0
                        for r in range(2):
                            mt, w0 = msg_t[r]
                            for k in range(int(W[b, r])):
                                wg = int(woff[b, r]) + k          # global window
                                wl = wg - w0                       # window in chunk
                                P = pwp.tile([128, 128], bf16, tag="P", name="P")
                                nc.vector.tensor_scalar(
                                    out=P[:],
                                    in0=iota_t[:],
                                    scalar1=dslot_t[:, wg : wg + 1],
                                    scalar2=None,
                                    op0=mybir.AluOpType.is_equal,
                                )
                                nc.tensor.matmul(
                                    pa[:],
                                    lhsT=mt[:, wl * DIM : (wl + 1) * DIM],
                                    rhs=P[:],
                                    start=(wi == 0),
                                    stop=(wi == nwin_b - 1),
                                )
                                wi += 1
                        # aggT scaled by 1/deg (psum -> sbuf fused)
                        aggT = wkp.tile([128, 128], f32, tag="aggT", name="aggT")
                        nc.vector.tensor_tensor(
                            out=aggT[:],
                            in0=pa[:],
                            in1=invdeg_t[:, b * BLK : (b + 1) * BLK],
                            op=mybir.AluOpType.mult,
                        )
                        # dense: out = aggT.T @ Wn + h.T.T @ Ws + 1 x bias
                        po = psB.tile([128, 128], f32, tag="out", name="outp")
                        nc.tensor.matmul(
                            po[:], lhsT=aggT[:],
                            rhs=wn_t[l][:],
                            start=True, stop=False,
                        )
                        nc.tensor.matmul(
                            po[:], lhsT=hT_cur[:, b * BLK : (b + 1) * BLK],
                            rhs=ws_t[l][:],
                            start=False, stop=False,
                        )
                        nc.tensor.matmul(
                            po[:], lhsT=ones_t[:],
                            rhs=bias_t[l][:],
                            start=False, stop=True,
                        )
                        rows = min(BLK, NPC - b * BLK)
                        if l < N_LAYERS - 1:
                            hnew = wkp.tile([128, 128], f32, tag="hnew", name="hnew")
                            nc.scalar.activation(
                                hnew[:], po[:], mybir.ActivationFunctionType.Relu
                            )
                            hbf = wkp.tile([128, 128], bf16, tag="hbf", name="hbf")
                            nc.vector.tensor_copy(out=hbf[:], in_=hnew[:])
                            nc.sync.dma_start(
                                out=hown[l][b * BLK : b * BLK + rows, :],
                                in_=hbf[:rows, :],
                            )
                            pt = psT.tile([128, 128], f32, tag="tr", name="tr")
                            nc.tensor.transpose(
                                out=pt[:], in_=hnew[:], identity=ident_t[:]
                            )
                            nc.vector.tensor_copy(
                                out=hT_next[:, b * BLK : (b + 1) * BLK], in_=pt[:]
                            )
                        else:
                            ho = wkp.tile([128, 128], f32, tag="hnew", name="hnew")
                            nc.vector.tensor_copy(out=ho[:], in_=po[:])
                            nc.sync.dma_start(
                                out=out_ext[b * BLK : b * BLK + rows, :],
                                in_=ho[:rows, :],
                            )
                if l < N_LAYERS - 1:
                    if os.environ.get("GNN_NO_CC", "0") == "1":
                        nc.sync.dma_start(
                            out=hg[l][:NPC, :], in_=hown[l][:, :]
                        )
                    else:
                        nc.gpsimd.collective_compute(
                            "AllGather",
                            mybir.AluOpType.bypass,
                            replica_groups=rg,
                            ins=[hown[l].ap()],
                            outs=[hg[l].ap()],
                        )
    nc.compile()
    return nc


def kernel(x, src, dst, W_self, W_neigh, b):
    from concourse.bass_utils import run_bass_kernel_spmd

    x = np.asarray(x, np.float32)
    W_self = np.asarray(W_self, np.float32)
    W_neigh = np.asarray(W_neigh, np.float32)
    b = np.asarray(b, np.float32)

    W, per_core = _preprocess(src, dst)
    nw_run = W.sum(axis=0)

    deg = np.bincount(np.asarray(dst).astype(np.int64), minlength=N_NODES)
    invdeg = (1.0 / np.maximum(deg, 1)).astype(np.float32)

    nc = _build_program(W, nw_run)

    xbf = x.astype(ml_dtypes.bfloat16)
    iota = np.tile(np.arange(128, dtype=np.float32), (128, 1))
    ident = np.eye(128, dtype=np.float32)
    ws_flat = W_self.reshape(N_LAYERS * DIM, DIM)
    wn_flat = W_neigh.reshape(N_LAYERS * DIM, DIM)

    in_maps = []
    for c in range(CORES):
        idx128, dslotT = per_core[c]
        inv_c = np.zeros(NPC_PAD, np.float32)
        inv_c[:NPC] = invdeg[c * NPC : (c + 1) * NPC]
        inv_bc = np.tile(inv_c, (128, 1))
        xT = np.zeros((DIM, NPC_PAD), np.float32)
        xT[:, :NPC] = x[c * NPC : (c + 1) * NPC].T
        in_maps.append(
            {
                "xbf": xbf,
                "xT": xT,
                "idx": idx128,
                "dslot": dslotT,
                "invdeg": inv_bc,
                "Wself": ws_flat,
                "Wneigh": wn_flat,
                "bias": b,
                "iota": iota,
                "ident": ident,
            }
        )

    trace = os.environ.get("GNN_TRACE", "0") == "1"
    if trace:
        try:
            import types

            import antenv

            if "antenv.axon_hooks" not in sys.modules:
                mod = types.ModuleType("antenv.axon_hooks")
                mod._HOOK = None

                def _set(h, _m=mod):
                    _m._HOOK = h

                def _get(_m=mod):
                    return _m._HOOK

                mod.set_axon_ntff_profile_hook = _set
                mod.get_axon_ntff_profile_hook = _get
                sys.modules["antenv.axon_hooks"] = mod
                antenv.axon_hooks = mod
            from trn_agent_boot.trn_boot import _ntff_profile_via_ctypes

            sys.modules["antenv.axon_hooks"].set_axon_ntff_profile_hook(
                _ntff_profile_via_ctypes("/opt/axon/libaxon_pjrt.so")
            )
        except Exception as e:  # profiling is best-effort
            print(f"ntff hook setup failed: {e}")
            trace = False
    res = run_bass_kernel_spmd(
        nc, in_maps, core_ids=list(range(CORES)), trace=trace
    )
    LAST_EXEC_NS[0] = res.exec_time_ns
    LAST_PROFILE[0] = res.profile_json

    out = np.concatenate([res.results[c]["out"] for c in range(CORES)], axis=0)
    return out.astype(np.float32)



# revision 6
# speedup vs baseline: 2.4519x; 2.4519x over previous
"""3-layer GraphSAGE (mean agg) on 8 trn2 NeuronCores.

Sharding: nodes data-parallel (6250/core, by dst owner), weights replicated.
Per core: edges with dst in its node range, sorted by (src-half, dst-block),
padded to core-invariant per-(block,run) window counts so one SPMD program
works for all cores. Neighbor features fetched by dma_gather (bf16 rows)
spread round-robin over 4 SWDGE queues so ring drains pipeline, aggregated
per 128-edge window by one-hot matmuls accumulating in PSUM (one-hot P
matrices built in bulk per gather group via broadcast-AP is_equal), then
fused dense layer (W_self, W_neigh, bias via K=1 matmul) in bf16.
AllGather of bf16 activations between layers.
"""

import os
import sys

sys.path.insert(0, "/opt/trn_rl_repo")

import numpy as np
import ml_dtypes

N_NODES = 50000
N_EDGES = 800000
DIM = 128
N_LAYERS = 3
CORES = 8
NPC = N_NODES // CORES          # 6250 nodes per core
BLK = 128
NBLK = (NPC + BLK - 1) // BLK   # 49 blocks (last has 106 valid rows)
NPC_PAD = NBLK * BLK            # 6272
HALF = N_NODES // 2             # 25000 (int16 gather index limit is 32767)
GB = 5                          # blocks per gather group
NQ = 4                          # SWDGE queues (ucode max)
CH = 1024                       # idx per gather inst (= ring capacity/queue)

LAST_EXEC_NS = [None]
LAST_PROFILE = [None]


def _preprocess(src, dst):
    """Host-side graph preprocessing. Returns per-core index/dslot arrays plus
    the shared (core-invariant) window schedule."""
    src = np.asarray(src).astype(np.int64)
    dst = np.asarray(dst).astype(np.int64)

    owner = dst // NPC
    run = (src >= HALF).astype(np.int64)
    dloc = dst - owner * NPC
    blk = dloc // BLK

    # counts[c, b, r]
    counts = np.zeros((CORES, NBLK, 2), np.int64)
    np.add.at(counts, (owner, blk, run), 1)
    # core-invariant window counts per (block, run)
    W = np.maximum(1, -(-counts.max(axis=0) // BLK))  # [NBLK, 2] ceil-div
    nw_run = W.sum(axis=0)                            # windows per run
    nw_total = int(nw_run.sum())

    per_core = []
    for c in range(CORES):
        sel = owner == c
        es, eb, er, ed = src[sel], blk[sel], run[sel], dloc[sel]
        order = np.lexsort((eb, er))
        es, eb, er, ed = es[order], eb[order], er[order], ed[order]

        idx_out = np.zeros(nw_total * BLK, np.int16)
        dslot_out = np.full(nw_total * BLK, 255.0, np.float32)
        wpos = 0
        epos = 0
        for r in range(2):
            for b in range(NBLK):
                cnt = int(counts[c, b, r])
                cap = int(W[b, r]) * BLK
                e0, e1 = epos, epos + cnt
                o0 = wpos * BLK
                idx_out[o0:o0 + cnt] = (es[e0:e1] - r * HALF).astype(np.int16)
                dslot_out[o0:o0 + cnt] = (ed[e0:e1] - b * BLK).astype(np.float32)
                # pads: idx 0 (valid row, gathered but zeroed by P)
                epos = e1
                wpos += int(W[b, r])
                assert cnt <= cap
        assert epos == len(es)

        # wrap indices in 16 partitions, replicate to 128 (one copy / Q7 core)
        wrapped = idx_out.reshape(-1, 16).T.copy()        # [16, nw_total*8]
        idx128 = np.tile(wrapped, (8, 1))                 # [128, nw_total*8]
        # dslotT: [128, nw_total]; column w = dslots of window w's 128 edges
        dslotT = dslot_out.reshape(nw_total, BLK).T.copy()
        per_core.append((idx128, dslotT))

    return W, per_core


def _build_program(W, nw_run):
    import concourse.bass as bass
    import concourse.mybir as mybir
    import concourse.tile as tile
    from concourse import bacc

    f32 = mybir.dt.float32
    bf16 = mybir.dt.bfloat16
    i16 = mybir.dt.int16

    nw_total = int(W.sum())

    nc = bacc.Bacc(
        "TRN2",
        target_bir_lowering=False,
        num_devices=CORES,
        num_swdge_queues=NQ,
    )

    # I/O
    xbf = nc.declare_dram_parameter("xbf", [N_NODES, DIM], bf16, isOutput=False)
    xT_in = nc.declare_dram_parameter("xT", [DIM, NPC_PAD], bf16, isOutput=False)
    idx_in = nc.declare_dram_parameter("idx", [128, nw_total * 8], i16, isOutput=False)
    dslot_in = nc.declare_dram_parameter("dslot", [128, nw_total], f32, isOutput=False)
    invdeg_in = nc.declare_dram_parameter("invdeg", [1, NPC_PAD], f32, isOutput=False)
    ws_in = nc.declare_dram_parameter("Wself", [N_LAYERS * DIM, DIM], bf16, isOutput=False)
    wn_in = nc.declare_dram_parameter("Wneigh", [N_LAYERS * DIM, DIM], bf16, isOutput=False)
    b_in = nc.declare_dram_parameter("bias", [N_LAYERS, DIM], bf16, isOutput=False)
    iota_in = nc.declare_dram_parameter("iota", [128, 128], f32, isOutput=False)
    ident_in = nc.declare_dram_parameter("ident", [128, 128], bf16, isOutput=False)
    out_ext = nc.declare_dram_parameter("out", [NPC, DIM], f32, isOutput=True)

    # internal DRAM for collectives
    hown = [nc.dram_tensor(f"hown{l}", [NPC, DIM], bf16) for l in range(2)]
    hg = [
        nc.dram_tensor(f"hg{l}", [N_NODES, DIM], bf16, addr_space="Shared")
        for l in range(2)
    ]
    rg = [list(range(CORES))]

    # group structure: groups of GB blocks
    groups = [list(range(s, min(s + GB, NBLK))) for s in range(0, NBLK, GB)]
    # per-run window offset of each (b, r) in the stream (run-major, block order)
    woff = np.zeros((NBLK, 2), np.int64)
    w = 0
    for r in range(2):
        for b in range(NBLK):
            woff[b, r] = w
            w += int(W[b, r])
    assert w == nw_total

    qctr = [0]  # SWDGE queue round-robin across all gathers

    with tile.TileContext(nc) as tc:
        with (
            tc.tile_pool(name="persist", bufs=1) as pp,
            tc.tile_pool(name="msg", bufs=3) as msgp,
            tc.tile_pool(name="pwin", bufs=2) as pwp,
            tc.tile_pool(name="work", bufs=4) as wkp,
            tc.tile_pool(name="psA", bufs=2, space="PSUM") as psA,
            tc.tile_pool(name="psB", bufs=2, space="PSUM") as psB,
            tc.tile_pool(name="psT", bufs=2, space="PSUM") as psT,
        ):
            # --- persistent SBUF loads ---
            def load(shape, dt, src_ap, tag):
                t = pp.tile(shape, dt, tag=tag, name=tag)
                nc.sync.dma_start(out=t[:], in_=src_ap)
                return t

            idx_t = load([128, nw_total * 8], i16, idx_in[:, :], "idx")
            dslot_t = load([128, nw_total], f32, dslot_in[:, :], "dslot")
            invdeg_t = pp.tile([128, NPC_PAD], f32, tag="invdeg", name="invdeg")
            nc.sync.dma_start(
                out=invdeg_t[:],
                in_=invdeg_in[0:1, :].broadcast_to([128, NPC_PAD]),
            )
            iota_t = load([128, 128], f32, iota_in[:, :], "iota")
            ident_t = load([128, 128], bf16, ident_in[:, :], "ident")
            ws_t = [
                load([128, DIM], bf16, ws_in[l * DIM : (l + 1) * DIM, :], f"ws{l}")
                for l in range(N_LAYERS)
            ]
            wn_t = [
                load([128, DIM], bf16, wn_in[l * DIM : (l + 1) * DIM, :], f"wn{l}")
                for l in range(N_LAYERS)
            ]
            bias_t = [
                load([1, DIM], bf16, b_in[l : l + 1, :], f"bias{l}")
                for l in range(N_LAYERS)
            ]
            ones_t = pp.tile([1, 128], bf16, tag="ones", name="ones")
            nc.vector.memset(ones_t[:], 1.0)

            # h transposed (bf16) for the self path; ping-pong buffers
            hT = [
                load([DIM, NPC_PAD], bf16, xT_in[:, :], "hT0"),
                pp.tile([DIM, NPC_PAD], bf16, tag="hT1", name="hT1"),
            ]

            for l in range(N_LAYERS):
                src_dram = xbf if l == 0 else hg[l - 1]
                hT_cur = hT[l % 2]
                hT_next = hT[(l + 1) % 2]
                for g, blocks in enumerate(groups):
                    # two gathers (one per src half) covering this group's windows
                    msg_t = []
                    P_t = []
                    for r in range(2):
                        w0 = int(woff[blocks[0], r])
                        nw = int(sum(W[b, r] for b in blocks))
                        nidx = nw * BLK
                        mt = msgp.tile([128, nw * DIM], bf16, tag=f"msg{r}", name=f"msg{r}")
                        for s0 in range(0, nidx, CH):
                            n = min(CH, nidx - s0)
                            nc.gpsimd.dma_gather(
                                out_ap=mt[:, s0 : s0 + n].rearrange(
                                    "p (w e) -> p w e", e=DIM
                                ),
                                in_ap=src_dram[r * HALF : (r + 1) * HALF, :],
                                idxs_ap=idx_t[
                                    :, w0 * 8 + s0 // 16 : w0 * 8 + (s0 + n) // 16
                                ],
                                num_idxs=n,
                                num_idxs_reg=n,
                                elem_size=DIM,
                                elem_step=DIM,
                                queue_num=qctr[0] % NQ,
                            )
                            qctr[0] += 1
                        msg_t.append((mt, w0))
                        # one-hot P for all nw windows in one DVE op:
                        # P[e, w, slot] = (iota[slot] == dslot[e, w])
                        Pw = pwp.tile([128, nw * BLK], bf16, tag=f"P{r}", name=f"P{r}")
                        nc.vector.tensor_tensor(
                            out=Pw[:].rearrange("p (w e) -> p w e", e=BLK),
                            in0=iota_t[:].unsqueeze(1).broadcast_to([128, nw, BLK]),
                            in1=dslot_t[:, w0 : w0 + nw]
                            .unsqueeze(2)
                            .broadcast_to([128, nw, BLK]),
                            op=mybir.AluOpType.is_equal,
                        )
                        P_t.append(Pw)

                    for b in blocks:
                        pa = psA.tile([128, 128], f32, tag="agg", name="agg")
                        nwin_b = int(W[b, 0] + W[b, 1])
                        wi = 0
                        for r in range(2):
                            mt, w0 = msg_t[r]
                            Pw = P_t[r]
                            for k in range(int(W[b, r])):
                                wg = int(woff[b, r]) + k          # global window
                                wl = wg - w0                       # window in chunk
                                nc.tensor.matmul(
                                    pa[:],
                                    lhsT=mt[:, wl * DIM : (wl + 1) * DIM],
                                    rhs=Pw[:, wl * BLK : (wl + 1) * BLK],
                                    start=(wi == 0),
                                    stop=(wi == nwin_b - 1),
                                )
                                wi += 1
                        # aggT scaled by 1/deg (psum -> sbuf fused, bf16 out)
                        aggT = wkp.tile([128, 128], bf16, tag="aggT", name="aggT")
                        nc.vector.tensor_tensor(
                            out=aggT[:],
                            in0=pa[:],
                            in1=invdeg_t[:, b * BLK : (b + 1) * BLK],
                            op=mybir.AluOpType.mult,
                        )
                        # dense: out = aggT.T @ Wn + h.T.T @ Ws + 1 x bias
                        po = psB.tile([128, 128], f32, tag="out", name="outp")
                        nc.tensor.matmul(
                            po[:], lhsT=aggT[:],
                            rhs=wn_t[l][:],
                            start=True, stop=False,
                        )
                        nc.tensor.matmul(
                            po[:], lhsT=hT_cur[:, b * BLK : (b + 1) * BLK],
                            rhs=ws_t[l][:],
                            start=False, stop=False,
                        )
                        nc.tensor.matmul(
                            po[:], lhsT=ones_t[:],
                            rhs=bias_t[l][:],
                            start=False, stop=True,
                        )
                        rows = min(BLK, NPC - b * BLK)
                        if l < N_LAYERS - 1:
                            hbf = wkp.tile([128, 128], bf16, tag="hbf", name="hbf")
                            nc.scalar.activation(
                                hbf[:], po[:], mybir.ActivationFunctionType.Relu
                            )
                            nc.sync.dma_start(
                                out=hown[l][b * BLK : b * BLK + rows, :],
                                in_=hbf[:rows, :],
                            )
                            pt = psT.tile([128, 128], bf16, tag="tr", name="tr")
                            nc.tensor.transpose(
                                out=pt[:], in_=hbf[:], identity=ident_t[:]
                            )
                            nc.vector.tensor_copy(
                                out=hT_next[:, b * BLK : (b + 1) * BLK], in_=pt[:]
                            )
                        else:
                            ho = wkp.tile([128, 128], f32, tag="hnew", name="hnew")
                            nc.vector.tensor_copy(out=ho[:], in_=po[:])
                            nc.sync.dma_start(
                                out=out_ext[b * BLK : b * BLK + rows, :],
                                in_=ho[:rows, :],
                            )
                if l < N_LAYERS - 1:
                    if os.environ.get("GNN_NO_CC", "0") == "1":
                        nc.sync.dma_start(
                            out=hg[l][:NPC, :], in_=hown[l][:, :]
                        )
                    else:
                        nc.gpsimd.collective_compute(
                            "AllGather",
                            mybir.AluOpType.bypass,
                            replica_groups=rg,
                            ins=[hown[l].ap()],
                            outs=[hg[l].ap()],
                        )
    nc.compile()
    return nc


def kernel(x, src, dst, W_self, W_neigh, b):
    from concourse.bass_utils import run_bass_kernel_spmd

    x = np.asarray(x, np.float32)
    W_self = np.asarray(W_self, np.float32)
    W_neigh = np.asarray(W_neigh, np.float32)
    b = np.asarray(b, np.float32)

    W, per_core = _preprocess(src, dst)
    nw_run = W.sum(axis=0)

    deg = np.bincount(np.asarray(dst).astype(np.int64), minlength=N_NODES)
    invdeg = (1.0 / np.maximum(deg, 1)).astype(np.float32)

    nc = _build_program(W, nw_run)

    xbf = x.astype(ml_dtypes.bfloat16)
    iota = np.tile(np.arange(128, dtype=np.float32), (128, 1))
    ident = np.eye(128, dtype=ml_dtypes.bfloat16)
    ws_flat = W_self.reshape(N_LAYERS * DIM, DIM).astype(ml_dtypes.bfloat16)
    wn_flat = W_neigh.reshape(N_LAYERS * DIM, DIM).astype(ml_dtypes.bfloat16)
    b_bf = b.astype(ml_dtypes.bfloat16)

    in_maps = []
    for c in range(CORES):
        idx128, dslotT = per_core[c]
        inv_c = np.zeros((1, NPC_PAD), np.float32)
        inv_c[0, :NPC] = invdeg[c * NPC : (c + 1) * NPC]
        xT = np.zeros((DIM, NPC_PAD), ml_dtypes.bfloat16)
        xT[:, :NPC] = x[c * NPC : (c + 1) * NPC].T
        in_maps.append(
            {
                "xbf": xbf,
                "xT": xT,
                "idx": idx128,
                "dslot": dslotT,
                "invdeg": inv_c,
                "Wself": ws_flat,
                "Wneigh": wn_flat,
                "bias": b_bf,
                "iota": iota,
                "ident": ident,
            }
        )

    trace = os.environ.get("GNN_TRACE", "0") == "1"
    if trace:
        try:
            import types

            import antenv

            if "antenv.axon_hooks" not in sys.modules:
                mod = types.ModuleType("antenv.axon_hooks")
                mod._HOOK = None

                def _set(h, _m=mod):
                    _m._HOOK = h

                def _get(_m=mod):
                    return _m._HOOK

                mod.set_axon_ntff_profile_hook = _set
                mod.get_axon_ntff_profile_hook = _get
                sys.modules["antenv.axon_hooks"] = mod
                antenv.axon_hooks = mod
            from trn_agent_boot.trn_boot import _ntff_profile_via_ctypes

            sys.modules["antenv.axon_hooks"].set_axon_ntff_profile_hook(
                _ntff_profile_via_ctypes("/opt/axon/libaxon_pjrt.so")
            )
        except Exception as e:  # profiling is best-effort
            print(f"ntff hook setup failed: {e}")
            trace = False
    res = run_bass_kernel_spmd(
        nc, in_maps, core_ids=list(range(CORES)), trace=trace
    )
    LAST_EXEC_NS[0] = res.exec_time_ns
    LAST_PROFILE[0] = res.profile_json

    out = np.concatenate([res.results[c]["out"] for c in range(CORES)], axis=0)
    return out.astype(np.float32)


# revision 7
# speedup vs baseline: 2.5905x; 1.0565x over previous
"""3-layer GraphSAGE (mean agg) on 8 trn2 NeuronCores.

Sharding: nodes data-parallel (6250/core, by dst owner), weights replicated.
Per core: edges with dst in its node range, sorted by (src-half, dst-block),
padded to core-invariant per-(block,run) window counts so one SPMD program
works for all cores. Neighbor features fetched by dma_gather (bf16 rows)
spread round-robin over 4 SWDGE queues so ring drains pipeline, aggregated
per 128-edge window by one-hot matmuls accumulating in PSUM (one-hot P
matrices built in bulk per gather group via broadcast-AP is_equal), then
fused dense layer (W_self, W_neigh, bias via K=1 matmul) in bf16.
AllGather of bf16 activations between layers.
"""

import os
import sys

sys.path.insert(0, "/opt/trn_rl_repo")

import numpy as np
import ml_dtypes

N_NODES = 50000
N_EDGES = 800000
DIM = 128
N_LAYERS = 3
CORES = 8
NPC = N_NODES // CORES          # 6250 nodes per core
BLK = 128
NBLK = (NPC + BLK - 1) // BLK   # 49 blocks (last has 106 valid rows)
NPC_PAD = NBLK * BLK            # 6272
HALF = N_NODES // 2             # 25000 (int16 gather index limit is 32767)
GB = 5                          # blocks per gather group
NQ = 4                          # SWDGE queues (ucode max)
CH = 1024                       # idx per gather inst (= ring capacity/queue)

LAST_EXEC_NS = [None]
LAST_PROFILE = [None]


def _preprocess(src, dst):
    """Host-side graph preprocessing. Returns per-core index/dslot arrays plus
    the shared (core-invariant) window schedule."""
    src = np.asarray(src).astype(np.int64)
    dst = np.asarray(dst).astype(np.int64)

    owner = dst // NPC
    run = (src >= HALF).astype(np.int64)
    dloc = dst - owner * NPC
    blk = dloc // BLK

    # counts[c, b, r]
    counts = np.zeros((CORES, NBLK, 2), np.int64)
    np.add.at(counts, (owner, blk, run), 1)
    # core-invariant window counts per (block, run)
    W = np.maximum(1, -(-counts.max(axis=0) // BLK))  # [NBLK, 2] ceil-div
    nw_run = W.sum(axis=0)                            # windows per run
    nw_total = int(nw_run.sum())

    per_core = []
    for c in range(CORES):
        sel = owner == c
        es, eb, er, ed = src[sel], blk[sel], run[sel], dloc[sel]
        order = np.lexsort((eb, er))
        es, eb, er, ed = es[order], eb[order], er[order], ed[order]

        idx_out = np.zeros(nw_total * BLK, np.int16)
        dslot_out = np.full(nw_total * BLK, 255.0, np.float32)
        wpos = 0
        epos = 0
        for r in range(2):
            for b in range(NBLK):
                cnt = int(counts[c, b, r])
                cap = int(W[b, r]) * BLK
                e0, e1 = epos, epos + cnt
                o0 = wpos * BLK
                idx_out[o0:o0 + cnt] = (es[e0:e1] - r * HALF).astype(np.int16)
                dslot_out[o0:o0 + cnt] = (ed[e0:e1] - b * BLK).astype(np.float32)
                # pads: idx 0 (valid row, gathered but zeroed by P)
                epos = e1
                wpos += int(W[b, r])
                assert cnt <= cap
        assert epos == len(es)

        # wrap indices in 16 partitions, replicate to 128 (one copy / Q7 core)
        wrapped = idx_out.reshape(-1, 16).T.copy()        # [16, nw_total*8]
        idx128 = np.tile(wrapped, (8, 1))                 # [128, nw_total*8]
        # dslotT: [128, nw_total]; column w = dslots of window w's 128 edges
        dslotT = dslot_out.reshape(nw_total, BLK).T.copy()
        per_core.append((idx128, dslotT))

    return W, per_core


def _build_program(W, nw_run):
    import concourse.bass as bass
    import concourse.mybir as mybir
    import concourse.tile as tile
    from concourse import bacc

    f32 = mybir.dt.float32
    bf16 = mybir.dt.bfloat16
    i16 = mybir.dt.int16

    nw_total = int(W.sum())

    nc = bacc.Bacc(
        "TRN2",
        target_bir_lowering=False,
        num_devices=CORES,
        num_swdge_queues=NQ,
        dynamic_dma_scratch_size=int(os.environ.get("GNN_SCRATCH", "32768")),
    )

    # I/O
    xbf = nc.declare_dram_parameter("xbf", [N_NODES, DIM], bf16, isOutput=False)
    xT_in = nc.declare_dram_parameter("xT", [DIM, NPC_PAD], bf16, isOutput=False)
    idx_in = nc.declare_dram_parameter("idx", [128, nw_total * 8], i16, isOutput=False)
    dslot_in = nc.declare_dram_parameter("dslot", [128, nw_total], f32, isOutput=False)
    invdeg_in = nc.declare_dram_parameter("invdeg", [1, NPC_PAD], f32, isOutput=False)
    ws_in = nc.declare_dram_parameter("Wself", [N_LAYERS * DIM, DIM], bf16, isOutput=False)
    wn_in = nc.declare_dram_parameter("Wneigh", [N_LAYERS * DIM, DIM], bf16, isOutput=False)
    b_in = nc.declare_dram_parameter("bias", [N_LAYERS, DIM], bf16, isOutput=False)
    iota_in = nc.declare_dram_parameter("iota", [128, 128], f32, isOutput=False)
    ident_in = nc.declare_dram_parameter("ident", [128, 128], bf16, isOutput=False)
    out_ext = nc.declare_dram_parameter("out", [NPC, DIM], f32, isOutput=True)

    # internal DRAM for collectives
    hown = [nc.dram_tensor(f"hown{l}", [NPC, DIM], bf16) for l in range(2)]
    hg = [
        nc.dram_tensor(f"hg{l}", [N_NODES, DIM], bf16, addr_space="Shared")
        for l in range(2)
    ]
    rg = [list(range(CORES))]

    # group structure: groups of GB blocks
    groups = [list(range(s, min(s + GB, NBLK))) for s in range(0, NBLK, GB)]
    # per-run window offset of each (b, r) in the stream (run-major, block order)
    woff = np.zeros((NBLK, 2), np.int64)
    w = 0
    for r in range(2):
        for b in range(NBLK):
            woff[b, r] = w
            w += int(W[b, r])
    assert w == nw_total

    qctr = [0]  # SWDGE queue round-robin across all gathers

    with tile.TileContext(nc) as tc:
        with (
            tc.tile_pool(name="persist", bufs=1) as pp,
            tc.tile_pool(name="msg", bufs=3) as msgp,
            tc.tile_pool(name="pwin", bufs=2) as pwp,
            tc.tile_pool(name="work", bufs=4) as wkp,
            tc.tile_pool(name="psA", bufs=2, space="PSUM") as psA,
            tc.tile_pool(name="psB", bufs=2, space="PSUM") as psB,
            tc.tile_pool(name="psT", bufs=2, space="PSUM") as psT,
        ):
            # --- persistent SBUF loads ---
            def load(shape, dt, src_ap, tag):
                t = pp.tile(shape, dt, tag=tag, name=tag)
                nc.sync.dma_start(out=t[:], in_=src_ap)
                return t

            idx_t = load([128, nw_total * 8], i16, idx_in[:, :], "idx")
            dslot_t = load([128, nw_total], f32, dslot_in[:, :], "dslot")
            invdeg_t = pp.tile([128, NPC_PAD], f32, tag="invdeg", name="invdeg")
            nc.sync.dma_start(
                out=invdeg_t[:],
                in_=invdeg_in[0:1, :].broadcast_to([128, NPC_PAD]),
            )
            iota_t = load([128, 128], f32, iota_in[:, :], "iota")
            ident_t = load([128, 128], bf16, ident_in[:, :], "ident")
            ws_t = [
                load([128, DIM], bf16, ws_in[l * DIM : (l + 1) * DIM, :], f"ws{l}")
                for l in range(N_LAYERS)
            ]
            wn_t = [
                load([128, DIM], bf16, wn_in[l * DIM : (l + 1) * DIM, :], f"wn{l}")
                for l in range(N_LAYERS)
            ]
            bias_t = [
                load([1, DIM], bf16, b_in[l : l + 1, :], f"bias{l}")
                for l in range(N_LAYERS)
            ]
            ones_t = pp.tile([1, 128], bf16, tag="ones", name="ones")
            nc.vector.memset(ones_t[:], 1.0)

            # h transposed (bf16) for the self path; ping-pong buffers
            hT = [
                load([DIM, NPC_PAD], bf16, xT_in[:, :], "hT0"),
                pp.tile([DIM, NPC_PAD], bf16, tag="hT1", name="hT1"),
            ]

            for l in range(N_LAYERS):
                src_dram = xbf if l == 0 else hg[l - 1]
                hT_cur = hT[l % 2]
                hT_next = hT[(l + 1) % 2]
                for g, blocks in enumerate(groups):
                    # two gathers (one per src half) covering this group's windows
                    msg_t = []
                    P_t = []
                    for r in range(2):
                        w0 = int(woff[blocks[0], r])
                        nw = int(sum(W[b, r] for b in blocks))
                        nidx = nw * BLK
                        mt = msgp.tile([128, nw * DIM], bf16, tag=f"msg{r}", name=f"msg{r}")
                        for s0 in range(0, nidx, CH):
                            n = min(CH, nidx - s0)
                            nc.gpsimd.dma_gather(
                                out_ap=mt[:, s0 : s0 + n].rearrange(
                                    "p (w e) -> p w e", e=DIM
                                ),
                                in_ap=src_dram[r * HALF : (r + 1) * HALF, :],
                                idxs_ap=idx_t[
                                    :, w0 * 8 + s0 // 16 : w0 * 8 + (s0 + n) // 16
                                ],
                                num_idxs=n,
                                num_idxs_reg=n,
                                elem_size=DIM,
                                elem_step=DIM,
                                queue_num=qctr[0] % NQ,
                            )
                            qctr[0] += 1
                        msg_t.append((mt, w0))
                        # one-hot P for all nw windows in one DVE op:
                        # P[e, w, slot] = (iota[slot] == dslot[e, w])
                        Pw = pwp.tile([128, nw * BLK], bf16, tag=f"P{r}", name=f"P{r}")
                        nc.vector.tensor_tensor(
                            out=Pw[:].rearrange("p (w e) -> p w e", e=BLK),
                            in0=iota_t[:].unsqueeze(1).broadcast_to([128, nw, BLK]),
                            in1=dslot_t[:, w0 : w0 + nw]
                            .unsqueeze(2)
                            .broadcast_to([128, nw, BLK]),
                            op=mybir.AluOpType.is_equal,
                        )
                        P_t.append(Pw)

                    for b in blocks:
                        pa = psA.tile([128, 128], f32, tag="agg", name="agg")
                        nwin_b = int(W[b, 0] + W[b, 1])
                        wi = 0
                        for r in range(2):
                            mt, w0 = msg_t[r]
                            Pw = P_t[r]
                            for k in range(int(W[b, r])):
                                wg = int(woff[b, r]) + k          # global window
                                wl = wg - w0                       # window in chunk
                                nc.tensor.matmul(
                                    pa[:],
                                    lhsT=mt[:, wl * DIM : (wl + 1) * DIM],
                                    rhs=Pw[:, wl * BLK : (wl + 1) * BLK],
                                    start=(wi == 0),
                                    stop=(wi == nwin_b - 1),
                                )
                                wi += 1
                        # aggT scaled by 1/deg (psum -> sbuf fused, bf16 out)
                        aggT = wkp.tile([128, 128], bf16, tag="aggT", name="aggT")
                        nc.vector.tensor_tensor(
                            out=aggT[:],
                            in0=pa[:],
                            in1=invdeg_t[:, b * BLK : (b + 1) * BLK],
                            op=mybir.AluOpType.mult,
                        )
                        # dense: out = aggT.T @ Wn + h.T.T @ Ws + 1 x bias
                        po = psB.tile([128, 128], f32, tag="out", name="outp")
                        nc.tensor.matmul(
                            po[:], lhsT=aggT[:],
                            rhs=wn_t[l][:],
                            start=True, stop=False,
                        )
                        nc.tensor.matmul(
                            po[:], lhsT=hT_cur[:, b * BLK : (b + 1) * BLK],
                            rhs=ws_t[l][:],
                            start=False, stop=False,
                        )
                        nc.tensor.matmul(
                            po[:], lhsT=ones_t[:],
                            rhs=bias_t[l][:],
                            start=False, stop=True,
                        )
                        rows = min(BLK, NPC - b * BLK)
                        if l < N_LAYERS - 1:
                            hbf = wkp.tile([128, 128], bf16, tag="hbf", name="hbf")
                            nc.scalar.activation(
                                hbf[:], po[:], mybir.ActivationFunctionType.Relu
                            )
                            nc.sync.dma_start(
                                out=hown[l][b * BLK : b * BLK + rows, :],
                                in_=hbf[:rows, :],
                            )
                            pt = psT.tile([128, 128], bf16, tag="tr", name="tr")
                            nc.tensor.transpose(
                                out=pt[:], in_=hbf[:], identity=ident_t[:]
                            )
                            nc.vector.tensor_copy(
                                out=hT_next[:, b * BLK : (b + 1) * BLK], in_=pt[:]
                            )
                        else:
                            ho = wkp.tile([128, 128], f32, tag="hnew", name="hnew")
                            nc.vector.tensor_copy(out=ho[:], in_=po[:])
                            nc.sync.dma_start(
                                out=out_ext[b * BLK : b * BLK + rows, :],
                                in_=ho[:rows, :],
                            )
                if l < N_LAYERS - 1:
                    if os.environ.get("GNN_NO_CC", "0") == "1":
                        nc.sync.dma_start(
                            out=hg[l][:NPC, :], in_=hown[l][:, :]
                        )
                    else:
                        nc.gpsimd.collective_compute(
                            "AllGather",
                            mybir.AluOpType.bypass,
                            replica_groups=rg,
                            ins=[hown[l].ap()],
                            outs=[hg[l].ap()],
                        )
    nc.compile()
    return nc


def kernel(x, src, dst, W_self, W_neigh, b):
    from concourse.bass_utils import run_bass_kernel_spmd

    x = np.asarray(x, np.float32)
    W_self = np.asarray(W_self, np.float32)
    W_neigh = np.asarray(W_neigh, np.float32)
    b = np.asarray(b, np.float32)

    W, per_core = _preprocess(src, dst)
    nw_run = W.sum(axis=0)

    deg = np.bincount(np.asarray(dst).astype(np.int64), minlength=N_NODES)
    invdeg = (1.0 / np.maximum(deg, 1)).astype(np.float32)

    nc = _build_program(W, nw_run)

    xbf = x.astype(ml_dtypes.bfloat16)
    iota = np.tile(np.arange(128, dtype=np.float32), (128, 1))
    ident = np.eye(128, dtype=ml_dtypes.bfloat16)
    ws_flat = W_self.reshape(N_LAYERS * DIM, DIM).astype(ml_dtypes.bfloat16)
    wn_flat = W_neigh.reshape(N_LAYERS * DIM, DIM).astype(ml_dtypes.bfloat16)
    b_bf = b.astype(ml_dtypes.bfloat16)

    in_maps = []
    for c in range(CORES):
        idx128, dslotT = per_core[c]
        inv_c = np.zeros((1, NPC_PAD), np.float32)
        inv_c[0, :NPC] = invdeg[c * NPC : (c + 1) * NPC]
        xT = np.zeros((DIM, NPC_PAD), ml_dtypes.bfloat16)
        xT[:, :NPC] = x[c * NPC : (c + 1) * NPC].T
        in_maps.append(
            {
                "xbf": xbf,
                "xT": xT,
                "idx": idx128,
                "dslot": dslotT,
                "invdeg": inv_c,
                "Wself": ws_flat,
                "Wneigh": wn_flat,
                "bias": b_bf,
                "iota": iota,
                "ident": ident,
            }
        )

    trace = os.environ.get("GNN_TRACE", "0") == "1"
    if trace:
        try:
            import types

            import antenv

            if "antenv.axon_hooks" not in sys.modules:
                mod = types.ModuleType("antenv.axon_hooks")
                mod._HOOK = None

                def _set(h, _m=mod):
                    _m._HOOK = h

                def _get(_m=mod):
                    return _m._HOOK

                mod.set_axon_ntff_profile_hook = _set
                mod.get_axon_ntff_profile_hook = _get
                sys.modules["antenv.axon_hooks"] = mod
                antenv.axon_hooks = mod
            from trn_agent_boot.trn_boot import _ntff_profile_via_ctypes

            sys.modules["antenv.axon_hooks"].set_axon_ntff_profile_hook(
                _ntff_profile_via_ctypes("/opt/axon/libaxon_pjrt.so")
            )
        except Exception as e:  # profiling is best-effort
            print(f"ntff hook setup failed: {e}")
            trace = False
    res = run_bass_kernel_spmd(
        nc, in_maps, core_ids=list(range(CORES)), trace=trace
    )
    LAST_EXEC_NS[0] = res.exec_time_ns
    LAST_PROFILE[0] = res.profile_json

    out = np.concatenate([res.results[c]["out"] for c in range(CORES)], axis=0)
    return out.astype(np.float32)
